# revision 8
# baseline (speedup 1.0000x reference)
"""Multi-head attention (B=2, S=2048, E=1024, H=16, D=64) on 8 TRN2 cores.

Sharding: tensor-parallel over heads — 2 heads per core. Each core gets the
full hidden_states and its 384 w_qkv columns (q|k|v, 128 each), computes
attention for its 2 heads over both batches, and writes a [4096, 128] output
shard (its heads' output columns). Host concatenates shards on the feature
dim. No collectives needed.

All matmuls run in float32r (TF32-like PE mode: bf16 speed at ~2e-4 relative
error; measured on HW). PSUM accumulation is fp32.

Per-core pipeline:
  Phase A (per 512-token tile): DMA X rows -> PE-transpose 128x128 blocks to
  X^T -> QKV^T matmuls accumulating 8 e-chunks -> Q^T/K^T persisted in SBUF;
  V^T PE-transposed back to V-natural layout with an appended ones column.
  Phase B (per batch, per 1024-query group, both heads): S^T = K^T.T @ Q^T
  per 128-key block with the two heads packed into PE row groups 0-63/64-127
  (concurrent), exp on ACT with scale=1/sqrt(E) folded in, PV with the ones
  column producing the softmax denominator as PSUM row 64 (pipelined: each
  key block's probabilities are consumed right after its exp), PE-transpose
  of the [65, q] result back to [q, 65], reciprocal + per-row scale on DVE,
  DMA out. Output drains are deferred into the next group's key loop so the
  ACT engine (the bottleneck: ~134M exp elements total) never starves.

PSUM budget (8 banks): rotating pool 2 x [128,1024]f32 (4 banks) shared by
attention score tiles and phase-A staging; output pool 2 x [65,1024]f32
(4 banks) shared by PV accumulators and the output transpose tiles.
"""
import numpy as np

import concourse.bacc as bacc
import concourse.mybir as mybir
import concourse.tile as tile
from concourse.bass_utils import run_bass_kernel_spmd
from concourse.masks import make_identity

B, S, E = 2, 2048, 1024
H, D = 16, 64
T = B * S            # 4096 tokens
EC = E // 128        # 8 e-chunks
NT = T // 512        # 8 token tiles
SCALE = 1.0 / 32.0   # 1/sqrt(E), exact in fp32
F32 = mybir.dt.float32
R32 = mybir.dt.float32r  # matmul-operand tiles; engine writes perform the f32r rounding
EXP = mybir.ActivationFunctionType.Exp

_cache = {}


def build():
    if "nc" in _cache:
        return _cache["nc"]
    nc = bacc.Bacc("TRN2", target_bir_lowering=False, debug=False)

    x = nc.dram_tensor("x", [T, E], F32, kind="ExternalInput")
    w = nc.dram_tensor("w", [E, 3 * 128], F32, kind="ExternalInput")
    out = nc.dram_tensor("out", [T, 128], F32, kind="ExternalOutput")

    with tile.TileContext(nc) as tc:
        with (
            tc.tile_pool(name="persist", bufs=1) as persist,
            tc.tile_pool(name="xp", bufs=6) as xp,
            tc.tile_pool(name="xtp", bufs=2) as xtp,
            tc.tile_pool(name="wtp", bufs=6) as wtp,
            tc.tile_pool(name="sop", bufs=2) as sop,
            tc.tile_pool(name="outp", bufs=9) as outp,
            tc.tile_pool(name="rp", bufs=8) as rp,
            tc.tile_pool(name="vstg", bufs=2) as vstg,
            tc.tile_pool(name="rot", bufs=2, space="PSUM") as rot,
            tc.tile_pool(name="pop", bufs=2, space="PSUM") as pop,
        ):
            QT = persist.tile([128, T], R32)
            KT = persist.tile([128, T], R32)
            # V natural layout: [key-part, 128-key chunk, head, d | ones]
            V = persist.tile([128, 32, 2, 65], R32)
            wsb = persist.tile([128, EC, 384], R32)
            ident = persist.tile([128, 128], F32)

            ones = persist.tile([128, 1], F32)
            make_identity(nc, ident[:])
            nc.vector.memset(ones[:], 1.0)
            nc.vector.tensor_copy(V[:, :, :, 64:65], ones.to_broadcast([128, 32, 2, 1]))
            nc.sync.dma_start(wsb[:], w.rearrange("(c p) n -> p c n", p=128).bitcast(R32))

            def phase_a(tt):
                """QKV^T for tokens [tt*512, (tt+1)*512)."""
                xts = xtp.tile([128, EC, 512], R32, tag="xt")
                xtiles = []
                for i in range(4):
                    xt_ = xp.tile([128, E], F32, tag="x")
                    r0 = (tt * 4 + i) * 128
                    nc.sync.dma_start(xt_[:], x[r0:r0 + 128, :])
                    xtiles.append(xt_)
                for eb in range(EC):
                    pt = rot.tile([128, 512], F32, tag="u")
                    for i in range(4):
                        nc.tensor.transpose(
                            pt[:, i * 128:(i + 1) * 128],
                            xtiles[i][:, eb * 128:(eb + 1) * 128],
                            ident[:],
                        )
                    # split psum->sbuf copies between DVE and the (idle) ACT
                    if eb % 2 == 0:
                        nc.vector.tensor_copy(xts[:, eb, :], pt[:])
                    else:
                        nc.scalar.copy(xts[:, eb, :], pt[:])
                for m in range(3):  # q, k, v
                    pa = rot.tile([128, 512], F32, tag="u")
                    for eb in range(EC):
                        nc.tensor.matmul(
                            pa[:],
                            wsb[:, eb, m * 128:(m + 1) * 128],
                            xts[:, eb, :],
                            start=(eb == 0),
                            stop=(eb == EC - 1),
                        )
                    if m == 0:
                        nc.scalar.copy(QT[:, tt * 512:(tt + 1) * 512], pa[:])
                    elif m == 1:
                        nc.vector.tensor_copy(KT[:, tt * 512:(tt + 1) * 512], pa[:])
                    else:
                        vt = vstg.tile([128, 512], F32, tag="v")
                        nc.vector.tensor_copy(vt[:], pa[:])
                        for i in range(4):
                            g = tt * 4 + i  # global 128-key chunk id
                            pv = rot.tile([128, 128], F32, tag="u")
                            nc.tensor.transpose(
                                pv[:], vt[:, i * 128:(i + 1) * 128], ident[:]
                            )
                            # pv columns: [head0 d0..63 | head1 d0..63]
                            nc.vector.tensor_copy(V[:, g, :, 0:64], pv[:])

            def attention(b, qg, filler):
                """Both heads, queries [b*S + qg*1024, +1024).

                filler: iterator of zero-arg callbacks (previous group's
                output drain steps), pumped once per key block so they
                interleave with this group's ACT-paced score loop.
                Returns this group's drain-step generator.
                """
                q0 = b * S + qg * 1024
                po = {}
                wt_prev = None

                def emit_pv(kb, wt):
                    for h in (0, 1):
                        if kb == 0:
                            po[h] = pop.tile([65, 1024], F32, tag="o", name="po")
                        g = b * 16 + kb
                        for half in (0, 1):
                            nc.tensor.matmul(
                                po[h][:, half * 512:(half + 1) * 512],
                                V[:, g, h, :],
                                wt[h][:, half * 512:(half + 1) * 512],
                                start=(kb == 0),
                                stop=(kb == 15),
                            )

                for kb in range(16):
                    k0 = b * S + kb * 128
                    ps = {0: rot.tile([128, 1024], F32, tag="u", name="psA"),
                          1: rot.tile([128, 1024], F32, tag="u", name="psB")}
                    # heads packed into PE row groups 0-63 / 64-127: keep the
                    # two heads' matmuls adjacent so they run concurrently
                    for half in (0, 1):
                        for h in (0, 1):
                            hs = slice(h * 64, (h + 1) * 64)
                            nc.tensor.matmul(
                                ps[h][:, half * 512:(half + 1) * 512],
                                KT[hs, k0:k0 + 128],
                                QT[hs, q0 + half * 512:q0 + (half + 1) * 512],
                            )
                    wt = {}
                    for h in (0, 1):
                        wt[h] = wtp.tile([128, 1024], R32, tag="w", name="wt")
                        nc.scalar.activation(wt[h][:], ps[h][:], EXP, scale=SCALE)
                    for _ in range(2):
                        step = next(filler, None)
                        if step is not None:
                            step()
                    if wt_prev is not None:
                        emit_pv(kb - 1, wt_prev)
                    wt_prev = wt
                emit_pv(15, wt_prev)

                def drain_steps():
                    osb = {}
                    so = {}
                    for h in (0, 1):
                        def copy_out(h=h):
                            so[h] = sop.tile([65, 1024], F32, tag="so", name="so")
                            nc.vector.tensor_copy(so[h][:], po[h][:])
                        yield copy_out
                    for sub in range(8):
                        for h in (0, 1):
                            def one(h=h, sub=sub):
                                tr = rot.tile([128, 65], F32, tag="u", name="tr")
                                nc.tensor.transpose(
                                    tr[:],
                                    so[h][:, sub * 128:(sub + 1) * 128],
                                    ident[:65, :65],
                                )
                                r = rp.tile([128, 1], F32, tag="r")
                                nc.vector.reciprocal(r[:], tr[:, 64:65])
                                if sub not in osb:
                                    osb[sub] = outp.tile([128, 128], F32, tag="ob", name="osb")
                                nc.vector.tensor_scalar_mul(
                                    osb[sub][:, h * 64:(h + 1) * 64],
                                    tr[:, 0:64], r[:],
                                )
                                if h == 1:
                                    t0 = q0 + sub * 128
                                    nc.sync.dma_start(
                                        out[t0:t0 + 128, :], osb[sub][:]
                                    )
                            yield one

                return drain_steps()

            for tt in range(NT):
                phase_a(tt)
            filler = iter(())
            for b in (0, 1):
                for qg in (0, 1):
                    filler = attention(b, qg, filler)
            for step in filler:
                step()

    nc.compile()
    _cache["nc"] = nc
    return nc


def make_in_maps(hidden_states, w_qkv):
    x = np.ascontiguousarray(
        np.asarray(hidden_states, dtype=np.float32).reshape(T, E)
    )
    w = np.asarray(w_qkv, dtype=np.float32)
    in_maps = []
    for c in range(8):
        wc = np.concatenate(
            [w[:, m * 1024 + c * 128: m * 1024 + (c + 1) * 128] for m in range(3)],
            axis=1,
        )
        in_maps.append({"x": x, "w": np.ascontiguousarray(wc)})
    return in_maps


def kernel(hidden_states, w_qkv, **run_kwargs):
    nc = build()
    res = run_bass_kernel_spmd(
        nc, make_in_maps(hidden_states, w_qkv), core_ids=list(range(8)),
        **run_kwargs,
    )
    full = np.concatenate([res.results[c]["out"] for c in range(8)], axis=1)
    return full.reshape(B, S, E).astype(np.float32)
